# revision 26
# baseline (speedup 1.0000x reference)
"""Trainium2 Bass kernel for nn_RNN_Tensorized.

Math: in the reference model, layers 2 and 3 receive sigma == zeros, so their
bilinear terms vanish exactly: h3[l,b,:] = elu(b3[l,:]) for every batch row b,
independent of the layer-1 RNN scan. The output therefore collapses to

    out[b, l] = sigmoid( sum_h elu(b3[l,h]) * (Ws[l,h,1]-Ws[l,h,0])
                         + bs[l,1]-bs[l,0] )

which depends only on b3, Ws, bs and is identical across the batch dim. This
is exact algebra (holds for any input values), not an approximation.

Sharding: pure data parallelism over batch - each of the 8 cores computes the
(tiny) per-step vector f[64] and writes its own [1024, 64] batch shard.

Device pipeline (per core), v2. Key structural choices:
  * h-on-partitions layout: the packed input pk[65, 256] holds b3^T, W0^T,
    W1^T on rows 0..63 (row h, col l) and bs0^T / bs1^T side by side on row 64, so the
    h-reduction becomes a PE matmul against a broadcast ones column, landing
    d[l] replicated across all 128 psum partitions with l on the free dim -
    exactly the orientation the output store needs (no transpose step).
  * fp16 datapath: DVE tensor_scalar ops hit the 4x perf mode, matmuls run
    1 cycle/row instead of fp32's 4, and the store moves half the bytes.
    PSUM accumulation stays f32. fp16's 10 mantissa bits keep the error two
    orders below the 2e-2 gate; exp overflow to inf is immunized by the
    min(e^x, 1) clamp in the elu identity below.
  * sigmoid(d) = 0.5*tanh(d/2) + 0.5: Tanh lives in the same activation
    table set as Exp, so the single table load happens under the input DMA
    and never again; using Sigmoid directly would insert a 1283ns table
    switch mid-chain.
  * elu(x)+1 = max(min(e^x, 1), x+1) (exact, exp monotone). The +1 offset is
    corrected through the matmul accumulation: a first matmul with a -1
    column seeds psum with -sum_h wd + (bs1-bs0).
  * The constructor-emitted const-pool memsets, init all-engine barrier, and
    branch-register inits are stripped (see _strip_init_prologue): nothing
    in this kernel uses them and they serialize ~900ns before the input DMA
    could otherwise issue.
  * Semaphore waits are attached directly to the consuming instructions
    (_wait_ge) instead of standalone EventSemaphore instructions, removing
    ~70ns of sequencer work per cross-engine hop on the critical path.
"""

import numpy as np

import concourse.bass as bass
from concourse import mybir
from concourse.bass_utils import run_bass_kernel_spmd

N_CORES = 8
B, L, H = 8192, 64, 64
B_SHARD = B // N_CORES  # 1024
PKC = 256  # packed free dim: b3T | W0T | W1T | pad   (512B rows in fp16)
REP = 8  # output rows per psum partition

F32 = mybir.dt.float32
F16 = mybir.dt.float16
ALU = mybir.AluOpType
ACTF = mybir.ActivationFunctionType


def _strip_init_prologue(nc):
    """Drop the constructor-emitted const-pool memsets, the init all-engine
    barrier, and the branch-compare register inits from the entry block. We
    never use const_aps (activation biases are explicit APs), this kernel has
    no conditional branches, and every cross-engine dependency is explicitly
    semaphored, so none of it is needed. Saves ~900ns of serial prologue
    before the input DMA can issue."""
    blk = nc.m.functions[0].blocks[0]
    drop = (
        mybir.InstMemset,
        mybir.InstDrain,
        mybir.InstEventSemaphore,
        mybir.InstRegisterMove,
    )
    blk.instructions = [i for i in blk.instructions if not isinstance(i, drop)]


def _strip_end_barrier(nc):
    """Drop the Block-exit all-engine barrier (per-engine Drain + barrier
    EventSemaphore pairs) and each body's trailing branch to the now-empty
    end block (engines fall through block order to the same place). The
    stock barrier costs ~300ns of tail sequencer work; its completion
    guarantee (no engine retires before the store lands) is restored by the
    explicit fin_sem fence below at ~1/4 the cost -- without a fence, the
    host readback was observed to occasionally race the store DMA and
    return an all-zeros buffer."""
    for blk in nc.m.functions[0].blocks:
        blk.instructions = [
            i
            for i in blk.instructions
            if not (
                isinstance(i, mybir.InstDrain)
                or (
                    isinstance(i, mybir.InstEventSemaphore)
                    and i.name.startswith("barrier_")
                )
                or (
                    blk.name != "main"
                    and isinstance(i, mybir.InstUnconditionalBranch)
                )
            )
        ]


def build_kernel():
    nc = bass.Bass(enable_partition_id=False, monotonic_sem_count=0)
    _strip_init_prologue(nc)
    pk = nc.declare_dram_parameter("pk", [L + 1, PKC], F16, isOutput=False)
    out = nc.declare_dram_parameter("out", [B_SHARD, L], F16, isOutput=True)
    out_wide = out.rearrange("(p r) l -> p (r l)", r=REP)

    from contextlib import ExitStack

    with ExitStack() as ctx:
        tpk = ctx.enter_context(nc.sbuf_tensor([L + 1, PKC], F16))
        er = ctx.enter_context(nc.sbuf_tensor([L, H], F16))
        c = ctx.enter_context(nc.sbuf_tensor([L, H], F16))
        prod = ctx.enter_context(nc.sbuf_tensor([L, H], F16))
        cwb = ctx.enter_context(nc.sbuf_tensor([L + 1, H], F16))
        tb1 = ctx.enter_context(nc.sbuf_tensor([L, H], F16))
        th = ctx.enter_context(nc.sbuf_tensor([128, L], F16))
        wide = ctx.enter_context(nc.sbuf_tensor([128, REP * L], F16))
        onesb = ctx.enter_context(nc.sbuf_tensor([L, 1], F16))
        negones = ctx.enter_context(nc.sbuf_tensor([L + 1, 1], F16))
        zbias = ctx.enter_context(nc.sbuf_tensor([128, 1], F32))
        warm = ctx.enter_context(nc.sbuf_tensor([1, 1], F32))
        psum_d = ctx.enter_context(nc.psum_tensor([128, L], F32))

        zb_sem = ctx.enter_context(nc.semaphore("zb_sem"))
        in_sem = ctx.enter_context(nc.semaphore("in_sem"))
        er_sem = ctx.enter_context(nc.semaphore("er_sem"))
        prep2_sem = ctx.enter_context(nc.semaphore("prep2_sem"))
        pool_sem = ctx.enter_context(nc.semaphore("pool_sem"))
        prod_sem = ctx.enter_context(nc.semaphore("prod_sem"))
        d_sem = ctx.enter_context(nc.semaphore("d_sem"))
        th_sem = ctx.enter_context(nc.semaphore("th_sem"))
        wide_sem = ctx.enter_context(nc.semaphore("wide_sem"))
        out_sem = ctx.enter_context(nc.semaphore("out_sem"))
        fin_sem = ctx.enter_context(nc.semaphore("fin_sem"))
        block = ctx.enter_context(nc.Block())

        # packed views (row h, col l)
        tb3 = tpk[0:L, 0:H]
        tw0 = tpk[0:L, H : 2 * H]
        tw1 = tpk[0:L, 2 * H : 3 * H]
        bs0r = tpk[L : L + 1, 0:H]
        bs1r = tpk[L : L + 1, H : 2 * H]

        o1 = onesb[:, 0:1]
        ones_bc = bass.AP(tensor=o1.tensor, offset=o1.offset, ap=[o1.ap[0], [0, 128]])
        n1 = negones[:, 0:1]
        neg_bc = bass.AP(tensor=n1.tensor, offset=n1.offset, ap=[n1.ap[0], [0, 128]])
        # th replicated 8x along a zero-stride free dim: [128, (8), 64]
        t0 = th[:, :]
        th_rep = bass.AP(
            tensor=t0.tensor, offset=t0.offset, ap=[t0.ap[0], [0, REP], t0.ap[1]]
        )
        # wide viewed as [128, (r 8), (l 64)]
        w0 = wide[:, :]
        wide_rl = bass.AP(
            tensor=w0.tensor, offset=w0.offset, ap=[w0.ap[0], [L, REP], [1, L]]
        )

        # issue the input load from the entry block, before the per-engine
        # branch into the Block bodies: the DMA starts at t=0, not t=50
        nc.sync.dma_start(out=tpk[:], in_=pk[:]).then_inc(in_sem, 16)

        @block.sync
        def _(sp):
            sp.dma_start(out=out_wide, in_=wide[:, :])._wait_ge(wide_sem, 1).then_inc(
                out_sem, 16
            )
            # completion fence: no engine retires before the store lands
            sp.wait_ge(out_sem, 16).then_inc(fin_sem, 4)

        @block.gpsimd
        def _(g):
            # matmul constant columns, ready long before the first matmul
            g.memset(onesb[:], 1.0)
            g.memset(negones[:], -1.0).then_inc(pool_sem, 1)
            g.wait_ge(fin_sem, 1)

        @block.scalar
        def _(a):
            # prewarm the exp/tanh activation table while the input DMA flies
            a.activation(warm[:], warm[:], ACTF.Exp, bias=zbias[0:1, :])._wait_ge(
                zb_sem, 1
            )
            a.activation(er[:], tb3, ACTF.Exp, bias=zbias[0:L, :])._wait_ge(
                in_sem, 16
            ).then_inc(er_sem, 1)
            # th = tanh(d/2); sigmoid(d) = 0.5*th + 0.5 applied by DVE below
            a.activation(
                th[:], psum_d[:, :], ACTF.Tanh, scale=0.5, bias=zbias[:, :]
            )._wait_ge(d_sem, 1).then_inc(th_sem, 1)
            a.wait_ge(fin_sem, 1)

        @block.vector
        def _(v):
            v.memset(zbias[:], 0.0).then_inc(zb_sem, 1)
            # prep in the shadow of the er activation
            v.tensor_scalar(tb1[:], tb3, 1.0, None, ALU.add)._wait_ge(in_sem, 16)
            v.tensor_sub(cwb[0:L, :], tw1, tw0)
            v.tensor_sub(cwb[L : L + 1, :], bs0r, bs1r).then_inc(prep2_sem, 1)
            # c = max(min(e^x, 1), x+1) = elu(x) + 1  (exact: exp monotonic)
            v.scalar_tensor_tensor(
                c[:], er[:], 1.0, tb1[:], ALU.min, ALU.max
            )._wait_ge(er_sem, 1)
            v.tensor_mul(prod[:], c[:], cwb[0:L, :]).then_inc(prod_sem, 1)
            # wide[p, r*64+l] = (th[p,l] + 1) * 0.5 = sigmoid(d[l])
            v.tensor_scalar(wide_rl, th_rep, 1.0, 0.5, ALU.add, ALU.mult)._wait_ge(
                th_sem, 1
            ).then_inc(wide_sem, 1)
            v.wait_ge(fin_sem, 1)

        @block.tensor
        def _(pe):
            pe.wait_ge(pool_sem, 1)
            # psum[m, l]  = -sum_h wd[l,h] + (bs1-bs0)[l]   (constants seed)
            pe.matmul(
                psum_d[:, :], neg_bc, cwb[:, :], start=True, stop=False
            )._wait_ge(prep2_sem, 1)
            # psum[m, l] += sum_h (elu+1)[l,h] * wd[l,h]  ->  d[l], all m
            pe.matmul(
                psum_d[:, :], ones_bc, prod[:, :], start=False, stop=True
            )._wait_ge(prod_sem, 1).then_inc(d_sem, 1)
            pe.wait_ge(fin_sem, 1)

    _strip_end_barrier(nc)
    return nc


_NC_CACHE = None


def kernel(**inputs) -> np.ndarray:
    global _NC_CACHE
    b3 = np.asarray(inputs["b3"], dtype=np.float32)  # [L, H]
    Ws = np.asarray(inputs["Ws"], dtype=np.float32)  # [L, H, 2]
    bs = np.asarray(inputs["bs"], dtype=np.float32)  # [L, 2]

    packed = np.zeros((L + 1, PKC), dtype=np.float32)
    packed[0:L, 0:H] = b3.T
    packed[0:L, H : 2 * H] = Ws[:, :, 0].T
    packed[0:L, 2 * H : 3 * H] = Ws[:, :, 1].T
    packed[L, 0:H] = bs[:, 0]
    packed[L, H : 2 * H] = bs[:, 1]
    packed = packed.astype(np.float16)

    if _NC_CACHE is None:
        _NC_CACHE = build_kernel()
    in_maps = [{"pk": packed} for _ in range(N_CORES)]
    res = run_bass_kernel_spmd(_NC_CACHE, in_maps, core_ids=list(range(N_CORES)))
    return np.concatenate(
        [np.asarray(res.results[i]["out"]).astype(np.float32) for i in range(N_CORES)],
        axis=0,
    )


# revision 27
# speedup vs baseline: 1.0063x; 1.0063x over previous
"""Trainium2 Bass kernel for nn_RNN_Tensorized.

Math: in the reference model, layers 2 and 3 receive sigma == zeros, so their
bilinear terms vanish exactly: h3[l,b,:] = elu(b3[l,:]) for every batch row b,
independent of the layer-1 RNN scan. The output therefore collapses to

    out[b, l] = sigmoid( sum_h elu(b3[l,h]) * (Ws[l,h,1]-Ws[l,h,0])
                         + bs[l,1]-bs[l,0] )

which depends only on b3, Ws, bs and is identical across the batch dim. This
is exact algebra (holds for any input values), not an approximation.

Sharding: pure data parallelism over batch - each of the 8 cores computes the
(tiny) per-step vector f[64] and writes its own [1024, 64] batch shard.

Device pipeline (per core), v2. Key structural choices:
  * h-on-partitions layout: the packed input pk[65, 256] holds b3^T, W0^T,
    W1^T on rows 0..63 (row h, col l) and bs0^T / bs1^T side by side on row 64, so the
    h-reduction becomes a PE matmul against a broadcast ones column, landing
    d[l] replicated across all 128 psum partitions with l on the free dim -
    exactly the orientation the output store needs (no transpose step).
  * fp16 datapath: DVE tensor_scalar ops hit the 4x perf mode, matmuls run
    1 cycle/row instead of fp32's 4, and the store moves half the bytes.
    PSUM accumulation stays f32. fp16's 10 mantissa bits keep the error two
    orders below the 2e-2 gate; exp overflow to inf is immunized by the
    min(e^x, 1) clamp in the elu identity below.
  * sigmoid(d) = 0.5*tanh(d/2) + 0.5: Tanh lives in the same activation
    table set as Exp, so the single table load happens under the input DMA
    and never again; using Sigmoid directly would insert a 1283ns table
    switch mid-chain.
  * elu(x)+1 = max(min(e^x, 1), x+1) (exact, exp monotone). The +1 offset is
    corrected through the matmul accumulation: a first matmul with a -1
    column seeds psum with -sum_h wd + (bs1-bs0).
  * The constructor-emitted const-pool memsets, init all-engine barrier, and
    branch-register inits are stripped (see _strip_init_prologue): nothing
    in this kernel uses them and they serialize ~900ns before the input DMA
    could otherwise issue.
  * Semaphore waits are attached directly to the consuming instructions
    (_wait_ge) instead of standalone EventSemaphore instructions, removing
    ~70ns of sequencer work per cross-engine hop on the critical path.
"""

import numpy as np

import concourse.bass as bass
from concourse import mybir
from concourse.bass_utils import run_bass_kernel_spmd

N_CORES = 8
B, L, H = 8192, 64, 64
B_SHARD = B // N_CORES  # 1024
PKC = 256  # packed free dim: b3T | W0T | W1T | pad   (512B rows in fp16)
REP = 8  # output rows per psum partition

F32 = mybir.dt.float32
F16 = mybir.dt.float16
ALU = mybir.AluOpType
ACTF = mybir.ActivationFunctionType


def _strip_init_prologue(nc):
    """Drop the constructor-emitted const-pool memsets, the init all-engine
    barrier, and the branch-compare register inits from the entry block. We
    never use const_aps (activation biases are explicit APs), this kernel has
    no conditional branches, and every cross-engine dependency is explicitly
    semaphored, so none of it is needed. Saves ~900ns of serial prologue
    before the input DMA can issue."""
    blk = nc.m.functions[0].blocks[0]
    drop = (
        mybir.InstMemset,
        mybir.InstDrain,
        mybir.InstEventSemaphore,
        mybir.InstRegisterMove,
    )
    blk.instructions = [i for i in blk.instructions if not isinstance(i, drop)]


def _strip_end_barrier(nc):
    """Drop the Block-exit all-engine barrier (per-engine Drain + barrier
    EventSemaphore pairs) and each body's trailing branch to the now-empty
    end block (engines fall through block order to the same place). The
    stock barrier costs ~300ns of tail sequencer work; its completion
    guarantee (no engine retires before the store lands) is restored by the
    explicit fin_sem fence below at ~1/4 the cost -- without a fence, the
    host readback was observed to occasionally race the store DMA and
    return an all-zeros buffer."""
    for blk in nc.m.functions[0].blocks:
        blk.instructions = [
            i
            for i in blk.instructions
            if not (
                isinstance(i, mybir.InstDrain)
                or (
                    isinstance(i, mybir.InstEventSemaphore)
                    and i.name.startswith("barrier_")
                )
                or (
                    blk.name != "main"
                    and isinstance(i, mybir.InstUnconditionalBranch)
                )
            )
        ]


def build_kernel():
    nc = bass.Bass(enable_partition_id=False, monotonic_sem_count=0)
    _strip_init_prologue(nc)
    pk = nc.declare_dram_parameter("pk", [L + 1, PKC], F16, isOutput=False)
    out = nc.declare_dram_parameter("out", [B_SHARD, L], F16, isOutput=True)
    out_wide = out.rearrange("(p r) l -> p (r l)", r=REP)

    from contextlib import ExitStack

    with ExitStack() as ctx:
        tpk = ctx.enter_context(nc.sbuf_tensor([L + 1, PKC], F16))
        er = ctx.enter_context(nc.sbuf_tensor([L, H], F16))
        c = ctx.enter_context(nc.sbuf_tensor([L, H], F16))
        prod = ctx.enter_context(nc.sbuf_tensor([L, H], F16))
        cwb = ctx.enter_context(nc.sbuf_tensor([L + 1, H], F16))
        tb1 = ctx.enter_context(nc.sbuf_tensor([L, H], F16))
        th = ctx.enter_context(nc.sbuf_tensor([128, L], F16))
        wide = ctx.enter_context(nc.sbuf_tensor([128, REP * L], F16))
        onesb = ctx.enter_context(nc.sbuf_tensor([L, 1], F16))
        negones = ctx.enter_context(nc.sbuf_tensor([L + 1, 1], F16))
        zbias = ctx.enter_context(nc.sbuf_tensor([128, 1], F32))
        warm = ctx.enter_context(nc.sbuf_tensor([1, 1], F32))
        psum_d = ctx.enter_context(nc.psum_tensor([128, L], F32))

        zb_sem = ctx.enter_context(nc.semaphore("zb_sem"))
        in_sem = ctx.enter_context(nc.semaphore("in_sem"))
        er_sem = ctx.enter_context(nc.semaphore("er_sem"))
        prep2_sem = ctx.enter_context(nc.semaphore("prep2_sem"))
        pool_sem = ctx.enter_context(nc.semaphore("pool_sem"))
        prod_sem = ctx.enter_context(nc.semaphore("prod_sem"))
        d_sem = ctx.enter_context(nc.semaphore("d_sem"))
        th_sem = ctx.enter_context(nc.semaphore("th_sem"))
        wide_sem = ctx.enter_context(nc.semaphore("wide_sem"))
        fin_sem = ctx.enter_context(nc.semaphore("fin_sem"))
        block = ctx.enter_context(nc.Block())

        # packed views (row h, col l)
        tb3 = tpk[0:L, 0:H]
        tw0 = tpk[0:L, H : 2 * H]
        tw1 = tpk[0:L, 2 * H : 3 * H]
        bs0r = tpk[L : L + 1, 0:H]
        bs1r = tpk[L : L + 1, H : 2 * H]

        o1 = onesb[:, 0:1]
        ones_bc = bass.AP(tensor=o1.tensor, offset=o1.offset, ap=[o1.ap[0], [0, 128]])
        n1 = negones[:, 0:1]
        neg_bc = bass.AP(tensor=n1.tensor, offset=n1.offset, ap=[n1.ap[0], [0, 128]])
        # th replicated 8x along a zero-stride free dim: [128, (8), 64]
        t0 = th[:, :]
        th_rep = bass.AP(
            tensor=t0.tensor, offset=t0.offset, ap=[t0.ap[0], [0, REP], t0.ap[1]]
        )
        # wide viewed as [128, (r 8), (l 64)]
        w0 = wide[:, :]
        wide_rl = bass.AP(
            tensor=w0.tensor, offset=w0.offset, ap=[w0.ap[0], [L, REP], [1, L]]
        )

        # issue the input load from the entry block, before the per-engine
        # branch into the Block bodies: the DMA starts at t=0, not t=50
        nc.sync.dma_start(out=tpk[:], in_=pk[:]).then_inc(in_sem, 16)

        @block.sync
        def _(sp):
            # the store bumps fin_sem directly on completion; every engine
            # (including SP) parks on it in parallel as its last instruction,
            # so none retires before the store lands
            sp.dma_start(out=out_wide, in_=wide[:, :])._wait_ge(wide_sem, 1).then_inc(
                fin_sem, 16
            )
            sp.wait_ge(fin_sem, 16)

        @block.gpsimd
        def _(g):
            # matmul constant columns, ready long before the first matmul
            g.memset(onesb[:], 1.0)
            g.memset(negones[:], -1.0).then_inc(pool_sem, 1)
            g.wait_ge(fin_sem, 16)

        @block.scalar
        def _(a):
            # prewarm the exp/tanh activation table while the input DMA flies
            a.activation(warm[:], warm[:], ACTF.Exp, bias=zbias[0:1, :])._wait_ge(
                zb_sem, 1
            )
            a.activation(er[:], tb3, ACTF.Exp, bias=zbias[0:L, :])._wait_ge(
                in_sem, 16
            ).then_inc(er_sem, 1)
            # th = tanh(d/2); sigmoid(d) = 0.5*th + 0.5 applied by DVE below
            a.activation(
                th[:], psum_d[:, :], ACTF.Tanh, scale=0.5, bias=zbias[:, :]
            )._wait_ge(d_sem, 1).then_inc(th_sem, 1)
            a.wait_ge(fin_sem, 16)

        @block.vector
        def _(v):
            v.memset(zbias[:], 0.0).then_inc(zb_sem, 1)
            # prep in the shadow of the er activation
            v.tensor_scalar(tb1[:], tb3, 1.0, None, ALU.add)._wait_ge(in_sem, 16)
            v.tensor_sub(cwb[0:L, :], tw1, tw0)
            v.tensor_sub(cwb[L : L + 1, :], bs0r, bs1r).then_inc(prep2_sem, 1)
            # c = max(min(e^x, 1), x+1) = elu(x) + 1  (exact: exp monotonic)
            v.scalar_tensor_tensor(
                c[:], er[:], 1.0, tb1[:], ALU.min, ALU.max
            )._wait_ge(er_sem, 1)
            v.tensor_mul(prod[:], c[:], cwb[0:L, :]).then_inc(prod_sem, 1)
            # wide[p, r*64+l] = (th[p,l] + 1) * 0.5 = sigmoid(d[l])
            v.tensor_scalar(wide_rl, th_rep, 1.0, 0.5, ALU.add, ALU.mult)._wait_ge(
                th_sem, 1
            ).then_inc(wide_sem, 1)
            v.wait_ge(fin_sem, 16)

        @block.tensor
        def _(pe):
            pe.wait_ge(pool_sem, 1)
            # psum[m, l]  = -sum_h wd[l,h] + (bs1-bs0)[l]   (constants seed)
            pe.matmul(
                psum_d[:, :], neg_bc, cwb[:, :], start=True, stop=False
            )._wait_ge(prep2_sem, 1)
            # psum[m, l] += sum_h (elu+1)[l,h] * wd[l,h]  ->  d[l], all m
            pe.matmul(
                psum_d[:, :], ones_bc, prod[:, :], start=False, stop=True
            )._wait_ge(prod_sem, 1).then_inc(d_sem, 1)
            pe.wait_ge(fin_sem, 16)

    _strip_end_barrier(nc)
    return nc


_NC_CACHE = None


def kernel(**inputs) -> np.ndarray:
    global _NC_CACHE
    b3 = np.asarray(inputs["b3"], dtype=np.float32)  # [L, H]
    Ws = np.asarray(inputs["Ws"], dtype=np.float32)  # [L, H, 2]
    bs = np.asarray(inputs["bs"], dtype=np.float32)  # [L, 2]

    packed = np.zeros((L + 1, PKC), dtype=np.float32)
    packed[0:L, 0:H] = b3.T
    packed[0:L, H : 2 * H] = Ws[:, :, 0].T
    packed[0:L, 2 * H : 3 * H] = Ws[:, :, 1].T
    packed[L, 0:H] = bs[:, 0]
    packed[L, H : 2 * H] = bs[:, 1]
    packed = packed.astype(np.float16)

    if _NC_CACHE is None:
        _NC_CACHE = build_kernel()
    in_maps = [{"pk": packed} for _ in range(N_CORES)]
    res = run_bass_kernel_spmd(_NC_CACHE, in_maps, core_ids=list(range(N_CORES)))
    return np.concatenate(
        [np.asarray(res.results[i]["out"]).astype(np.float32) for i in range(N_CORES)],
        axis=0,
    )
